# revision 36
# baseline (speedup 1.0000x reference)
"""Multi-head attention (B=2, S=2048, D=1024, H=16, Hd=64) on 8 Trainium2
NeuronCores.

Sharding: 8 cores = (batch 2) x (head-quarter 4).  Core (b, hq) computes,
for batch b and heads hq*4..hq*4+3, the full-sequence partial output

    outp = (softmax-attention of its 4 heads over all 2048 q rows) @ Wo_part.T

and the host sums the four head-quarter partials per batch and adds bo.

Everything is bf16 on the wire and in SBUF (PSUM accumulates fp32).
Host-side layouts are all p-major so every DMA descriptor is >= 4KB:
  xC     [NJ, 128, KT, 512]   x[b].T j-chunked, 8KB runs
  w{q,k,v}P [128, KT, DPC]    W.T slice, 4KB runs
  woP    [128, 2, D]          Wo.T slice, 4KB runs
  maskP  [128, NSK, S]        keep-mask (mask[b,0]==0).T, 4KB runs/tile
  outp   [S, D]               bf16 partial, 2KB rows

DMA queues: the 8MB mask streams on the qAct HWDGE queue (issued while
ACT is still idle) in parallel with x/weights/outputs on qSP.

Execution is one software-pipelined slot stream over all 8 (c, j) units
(c = head pair, j = 512-wide q chunk), 16 s_k tiles each — no
per-unit pipeline drain.  Slot g: scores matmul pair -> exp (ScalarE,
the pacing engine) -> keep-mask multiply (VectorE, 2x bf16 mode);
attnV accumulates L slots behind.  Projection chains, softmax-
normalization multiplies, and phase-3 output blocks are emitted from a
deadline-ordered (EDF) work heap, at most a few sub-microsecond pieces
per slot, so the PE stays dense without stalling the exp stream.
Emission order IS dependency order for the tile framework, so every
piece carries an exclusive deadline (= first slot that consumes it)
checked at build time.

Head packing: a pair's two heads live on partition halves 0-63/64-127
so the pair's two scores matmuls run concurrently on disjoint PE row
groups.  V lands in v_aug [128, NSK, head*128 + (64 V | 64 ones)]; the
ones columns make the attnV matmul accumulate Z = sum(expm) into PSUM
rows 64..127 at no extra stream cost (the stream time is set by the
512 moving columns, not the stationary width).
"""

import sys

if "/opt/trn_rl_repo" not in sys.path:
    sys.path.insert(0, "/opt/trn_rl_repo")

import heapq

import numpy as np

B, S, D = 2, 2048, 1024
H, HD = 16, 64
NCORES = 8
HPC = 4  # heads per core
DPC = HPC * HD  # 256 head dims per core
KT = D // 128  # 8 contraction tiles
NSK = S // 128  # 16 s_k tiles
NJ = S // 512  # 4 q chunks
NC2 = HPC // 2  # 2 head pairs

_CACHE = {}


def _build():
    import concourse.bacc as bacc
    import concourse.mybir as mybir
    import concourse.tile as tile

    F32 = mybir.dt.float32
    BF16 = mybir.dt.bfloat16
    MULT = mybir.AluOpType.mult
    EXP = mybir.ActivationFunctionType.Exp

    nc = bacc.Bacc("TRN2", target_bir_lowering=False, debug=False)

    xC = nc.dram_tensor("xC", [NJ, 128, KT, 512], BF16, kind="ExternalInput")
    wqP = nc.dram_tensor("wqP", [128, KT, DPC], BF16, kind="ExternalInput")
    wkP = nc.dram_tensor("wkP", [128, KT, DPC], BF16, kind="ExternalInput")
    wvP = nc.dram_tensor("wvP", [128, KT, DPC], BF16, kind="ExternalInput")
    woP = nc.dram_tensor("woP", [128, 2, D], BF16, kind="ExternalInput")
    maskP = nc.dram_tensor("maskP", [128, NSK, S], BF16, kind="ExternalInput")
    outp = nc.dram_tensor("outp", [S, D], BF16, kind="ExternalOutput")

    with tile.TileContext(nc) as tc:
        with (
            tc.tile_pool(name="keep", bufs=1) as keep,
            tc.tile_pool(name="pnorm", bufs=1) as pnorm,
            tc.tile_pool(name="p3s", bufs=2) as p3s,
            tc.tile_pool(name="scp", bufs=2, space="PSUM") as scp,
            tc.tile_pool(name="opp", bufs=1, space="PSUM") as opp,
            tc.tile_pool(name="aux0", bufs=1, space="PSUM") as aux0,
            tc.tile_pool(name="aux1", bufs=1, space="PSUM") as aux1,
        ):
            # ---- persistent SBUF ----------------------------------------
            x_sb = keep.tile([128, NJ, KT, 512], BF16)  # 32KB/part, j-major
            wq_sb = keep.tile([128, KT, DPC], BF16)
            wk_sb = keep.tile([128, KT, DPC], BF16)
            wv_sb = keep.tile([128, KT, DPC], BF16)
            wo_sb = keep.tile([128, 2, D], BF16)
            qT_sb = keep.tile([128, NC2, S], BF16)
            kT_sb = keep.tile([128, NC2, S], BF16)
            v_aug = keep.tile([128, NSK, HPC * 128], BF16)  # 16KB/part
            mask01 = keep.tile([128, NSK, S], BF16)  # 64KB/part, 0/1
            out_cT = keep.tile([128, NC2, S], BF16)
            # manual rings: exp output (6 slots; pairs at even offsets stay
            # adjacent) and masked-exp pairs (4 x 2-slot tiles) so one DVE
            # multiply covers two slots' tiles.
            expt_ring = keep.tile([128, 6, 2, 512], BF16)  # 24KB/part
            expm_ring = keep.tile([128, 4, 2, 2, 512], BF16)  # 16KB/part

            nc.any.memset(v_aug[:], 1.0)

            # ---- DMAs ---------------------------------------------------
            # Dual HWDGE queues (two DMA engines pull ~400GB/s aggregate
            # vs ~350 for one).  qAct front-loads the exp-stream critical
            # path: wk/x0/wq then mask tiles 0-7; its ring-credit blocking
            # of the ACT sequencer retires by ~10us, before the first exp.
            # qSP carries the rest in need-order plus the outputs.
            def dma_mask(engine, lo, hi):
                engine.dma_start(out=mask01[:, lo:hi, :], in_=maskP[:, lo:hi, :])

            def dma_x(engine, jc):
                engine.dma_start(out=x_sb[:, jc, :, :], in_=xC[jc, :, :, :])

            nc.scalar.dma_start(out=wk_sb[:], in_=wkP[:])
            dma_x(nc.scalar, 0)
            nc.scalar.dma_start(out=wq_sb[:], in_=wqP[:])
            dma_mask(nc.scalar, 0, 4)
            dma_mask(nc.scalar, 4, 8)

            nc.sync.dma_start(out=wv_sb[:], in_=wvP[:])
            dma_x(nc.sync, 1)
            dma_x(nc.sync, 2)
            dma_x(nc.sync, 3)
            dma_mask(nc.sync, 8, 12)
            dma_mask(nc.sync, 12, 16)
            nc.sync.dma_start(out=wo_sb[:], in_=woP[:])

            # ---- projection chains --------------------------------------
            # Every chain-like work item is a list of pieces, each a
            # function of the PSUM pool it accumulates in.  A piece list is
            # always emitted in order within ONE lane (one pool), so the
            # pool's single buffer is reused strictly
            # alloc->mm...->evict->alloc in emission order.
            def chain_kq(w_sb, dst_sb, c, jk, npieces=4):
                st = {}
                bounds = [
                    (KT * p // npieces, KT * (p + 1) // npieces)
                    for p in range(npieces)
                ]

                def piece(pool, r0, r1):
                    if r0 == 0:
                        st["ps"] = pool.tile(
                            [128, 512], F32, tag="aux", name=f"ch_{c}_{jk}"
                        )
                    ps = st["ps"]
                    for t in range(r0, r1):
                        nc.tensor.matmul(
                            ps[:],
                            w_sb[:, t, c * 128 : (c + 1) * 128],
                            x_sb[:, jk, t, :],
                            start=(t == 0),
                            stop=(t == KT - 1),
                        )
                    if r1 == KT:
                        sl = slice(jk * 512, (jk + 1) * 512)
                        nc.vector.tensor_copy(dst_sb[:, c, sl], ps[:])

                return [
                    lambda pool, r0=r0, r1=r1: piece(pool, r0, r1)
                    for r0, r1 in bounds
                ]

            def chain_v(sb):
                st = {}
                jv, uv = divmod(sb * 128, 512)

                def half(pool, r0, r1):
                    if r0 == 0:
                        st["ps"] = pool.tile(
                            [128, 256], F32, tag="aux", name=f"chv_{sb}"
                        )
                    ps = st["ps"]
                    for t in range(r0, r1):
                        nc.tensor.matmul(
                            ps[:],
                            x_sb[:, jv, t, uv : uv + 128],
                            wv_sb[:, t, :],
                            start=(t == 0),
                            stop=(t == KT - 1),
                        )
                    if r1 == KT:
                        nc.vector.tensor_copy(
                            v_aug[:, sb, :]
                            .rearrange("p (h c2) -> p h c2", h=HPC)[:, :, 0:HD],
                            ps[:].rearrange("p (h c2) -> p h c2", h=HPC),
                        )

                return [
                    lambda pool: half(pool, 0, 4),
                    lambda pool: half(pool, 4, KT),
                ]

            # ---- phase-3 output blocks ----------------------------------
            def phase3_block(mm, out_engine):
                """One m-block as a 4-piece chain: (n half x cb step); the
                last piece of each n half evicts, the last DMAs the row."""
                msl = slice(mm * 128, (mm + 1) * 128)
                st = {}

                def step(pool, cb, n):
                    if cb == 0:
                        st["ps"] = pool.tile(
                            [128, 512], F32, tag="aux", name=f"p3_{mm}_{n}"
                        )
                    nc.tensor.matmul(
                        st["ps"][:],
                        out_cT[:, cb, msl],
                        wo_sb[:, cb, n * 512 : (n + 1) * 512],
                        start=(cb == 0),
                        stop=(cb == 1),
                    )
                    if cb == 1:
                        if n == 0:
                            st["ob"] = p3s.tile(
                                [128, 1024], BF16, tag="ob", name=f"ob_{mm}"
                            )
                        nc.vector.tensor_copy(
                            st["ob"][:, n * 512 : (n + 1) * 512], st["ps"][:]
                        )
                        if n == 1:
                            out_engine.dma_start(
                                out=outp[msl, :], in_=st["ob"][:]
                            )

                return [
                    lambda pool, cb=cb, n=n: step(pool, cb, n)
                    for n in range(2)
                    for cb in range(2)
                ]

            # ---- the pipelined slot stream ------------------------------
            # Hybrid unit order: units 0-2 need only prefix + light Q
            # chains; K(1,*) has a 45-slot runway to unit 3; phase-3 of
            # j=0/1/2 spreads from units 4/6/7 onward.
            L = 6  # attnV lookahead in slots
            UNITS = [(0, 0), (0, 1), (1, 0), (0, 2), (1, 1), (0, 3), (1, 2), (1, 3)]
            NU = len(UNITS)
            NSLOT = NU * NSK
            U_OF = {cj: u for u, cj in enumerate(UNITS)}

            # PE warm-up: a dozen throwaway matmuls on memset scratch keep
            # the PE pstate ramping while the first DMAs land, so the
            # prefix chains run at mid clock instead of 0.65GHz.
            warm = keep.tile([128, 512], BF16)
            nc.vector.memset(warm[:], 1.0)
            wps = aux0.tile([128, 512], F32, tag="aux", name="warm_ps")
            for _ in range(12):
                nc.tensor.matmul(
                    wps[:], warm[:, 0:128], warm[:], start=True, stop=True
                )

            # prefix: only what gates the first few slots; everything else
            # streams in as lane work during unit 0, whose pace is set by
            # mask-DMA arrival anyway.  Alternate pools so consecutive
            # chains overlap (eviction of one vs matmuls of the next).
            prefix = [
                (chain_kq(wk_sb, kT_sb, 0, 0, 1), aux1),
                (chain_kq(wq_sb, qT_sb, 0, 0, 1), aux0),
                (chain_v(0), aux1),
                (chain_v(1), aux0),
            ]
            for pieces, pool in prefix:
                for p in pieces:
                    p(pool)

            # Chain heap: (first_sdl, seq, ready_slot, true_dl, pieces)
            # where pieces = [(sdl, fn), ...].  Two lanes, one PSUM pool
            # each; a lane hosts one chain at a time and emits a piece only
            # once its scheduled slot arrives (or the chain's true deadline
            # forces a burst).  The per-piece schedule drip-feeds the PE
            # uniformly — a bunched chain blob at a dropped PE clock is
            # what stalled the exp stream ~6us every unit.  A chain must
            # finish emission at a slot < true_dl; asserted below.
            chains = []
            _seq = [0]

            def add(deadline, fns, ready=0, spacing=1, end_slack=1, true_dl=None):
                n = len(fns)
                pieces = [
                    (max(deadline - end_slack - spacing * (n - 1 - i), 0), fn)
                    for i, fn in enumerate(fns)
                ]
                heapq.heappush(
                    chains,
                    (
                        pieces[0][0],
                        _seq[0],
                        ready,
                        true_dl if true_dl is not None else deadline,
                        pieces,
                    ),
                )
                _seq[0] += 1

            # remaining V chains: v(sb) consumed by attnV(0, sb) at slot
            # sb+L (lane work precedes attnV within a slot -> dl sb+L+1)
            for sb in range(2, NSK):
                add(sb + L + 1, chain_v(sb))
            # K(0,jk) consumed by scores(0, 4jk) at slot 4jk
            for jk in range(1, 4):
                add(4 * jk, chain_kq(wk_sb, kT_sb, 0, jk))
            # Q and K(1,*) chains, scheduled into slots i=0..6 of the
            # preceding unit (i=7..15 carry the phase-3 drip; overlapping
            # both in one slot is what stalled the exp at i~12)
            for j in range(1, 4):
                add(NSK * U_OF[(0, j)], chain_kq(wq_sb, qT_sb, 0, j),
                    spacing=2, end_slack=10)
            u10 = U_OF[(1, 0)]
            add(NSK * u10, chain_kq(wq_sb, qT_sb, 1, 0), spacing=2, end_slack=10)
            for jk in range(4):
                add(NSK * u10 + 4 * jk, chain_kq(wk_sb, kT_sb, 1, jk),
                    spacing=2, end_slack=10)
            for j in range(1, 4):
                add(NSK * U_OF[(1, j)], chain_kq(wq_sb, qT_sb, 1, j),
                    spacing=2, end_slack=10)
            # phase-3 m-blocks: ready once both (0,j) and (1,j) norm
            # multiplies are emitted; no real deadline (drain at the end).
            # The drain chunk's outputs ride the by-then-idle qAct queue.
            for j in range(4):
                gate = max(U_OF[(0, j)], U_OF[(1, j)])
                ready = gate * NSK + NSK - 1 + L + 2
                out_engine = nc.scalar if gate == NU - 1 else nc.sync
                for m in range(4):
                    add(
                        ready + 4 * m + 4,
                        phase3_block(4 * j + m, out_engine),
                        ready=ready,
                        true_dl=10**6,
                    )

            out_ps = [None] * NU

            def emit_scores(g):
                u, i = divmod(g, NSK)
                c, j = UNITS[u]
                jsl = slice(j * 512, (j + 1) * 512)
                sc = scp.tile([128, 2, 512], F32, tag="sc", name=f"sc_{u}_{i}")
                for h2 in range(2):
                    hsl = slice(h2 * 64, (h2 + 1) * 64)
                    nc.tensor.matmul(
                        sc[:, h2, :],
                        kT_sb[hsl, c, i * 128 : (i + 1) * 128],
                        qT_sb[hsl, c, jsl],
                        start=True,
                        stop=True,
                    )
                nc.scalar.activation(
                    out=expt_ring[:, g % 6, :, :], in_=sc[:], func=EXP, scale=0.125
                )
                # keep-mask multiply: tiles 8/9 of each unit go to the
                # otherwise-idle Pool engine (2.1us there, consumed by
                # attnV 4+ slots later); the rest run as slot-pair
                # multiplies on the DVE.
                if i in (8, 9):
                    nc.gpsimd.tensor_tensor(
                        out=expm_ring[:, (g // 2) % 4, g % 2, :, :],
                        in0=expt_ring[:, g % 6, :, :],
                        in1=mask01[:, i, jsl][:, None, :].to_broadcast(
                            (128, 2, 512)
                        ),
                        op=MULT,
                    )
                elif g % 2 == 1:
                    r0 = (g - 1) % 6
                    nc.vector.tensor_tensor(
                        out=expm_ring[:, (g // 2) % 4, :, :, :],
                        in0=expt_ring[:, r0 : r0 + 2, :, :],
                        in1=mask01[:, i - 1 : i + 1, jsl][
                            :, :, None, :
                        ].to_broadcast((128, 2, 2, 512)),
                        op=MULT,
                    )

            def emit_attnv(g2):
                u, i = divmod(g2, NSK)
                c, j = UNITS[u]
                if i == 0:
                    out_ps[u] = opp.tile(
                        [128, 2, 512], F32, tag="ops", name=f"op_{u}"
                    )
                for h2 in range(2):
                    h = 2 * c + h2
                    nc.tensor.matmul(
                        out_ps[u][:, h2, :],
                        v_aug[:, i, h * 128 : (h + 1) * 128],
                        expm_ring[:, (g2 // 2) % 4, g2 % 2, h2, :],
                        start=(i == 0),
                        stop=(i == NSK - 1),
                    )

            pending_muls = {}

            def emit_norm_start(u, g):
                """Z reciprocal + gpsimd broadcast right after unit u's last
                attnV; the two DVE multiplies are emitted directly at slot
                g+1, before the next unit's first attnV (which reuses the
                single opp buffer)."""
                c, j = UNITS[u]
                jsl = slice(j * 512, (j + 1) * 512)
                # both heads' Z rows live on partition 64 (h2 is a free
                # dim), so one copy + one reciprocal covers the pair
                zrow = pnorm.tile([1, 2, 512], F32, tag="zrow", name=f"zw_{u}")
                nc.vector.tensor_copy(zrow[:], out_ps[u][64:65, :, :])
                zr1 = pnorm.tile([1, 2, 512], F32, tag="zr1", name=f"z1_{u}")
                nc.vector.reciprocal_approx_fast(out=zr1[:], in_=zrow[:])
                muls = []
                for h2 in range(2):
                    zr = pnorm.tile([64, 512], F32, tag="zr", name=f"zr_{u}_{h2}")
                    nc.gpsimd.partition_broadcast(zr[:], zr1[:, h2, :])

                    def mul(u=u, c=c, h2=h2, jsl=jsl, zr=zr):
                        nc.vector.tensor_tensor(
                            out=out_cT[h2 * 64 : (h2 + 1) * 64, c, jsl],
                            in0=out_ps[u][0:64, h2, :],
                            in1=zr[:],
                            op=MULT,
                        )

                    muls.append(mul)
                pending_muls[g + 1] = muls

            norm_slot = {u * NSK + NSK - 1 + L: u for u in range(NU)}

            lanes = [
                {"pool": aux0, "pieces": [], "true_dl": 0},
                {"pool": aux1, "pieces": [], "true_dl": 0},
            ]

            def lane_work(g):
                deferred = []
                for lane in lanes:
                    emitted = 0
                    while emitted < 3:
                        if not lane["pieces"]:
                            got = None
                            while chains:
                                item = heapq.heappop(chains)
                                if item[2] > g:  # not ready yet
                                    deferred.append(item)
                                    continue
                                got = item
                                break
                            if got is None:
                                break
                            _sdl, _sq, _rdy, true_dl, pieces = got
                            lane["pieces"] = pieces
                            lane["true_dl"] = true_dl
                        # emit the head piece if its slot has come, or the
                        # chain's true deadline forces a burst
                        sdl, fn = lane["pieces"][0]
                        urgent = lane["true_dl"] <= g + 1 + len(lane["pieces"])
                        if sdl > g and not urgent:
                            break
                        lane["pieces"].pop(0)
                        fn(lane["pool"])
                        emitted += 1
                        if not lane["pieces"]:
                            assert lane["true_dl"] > g, (
                                f"chain past deadline {lane['true_dl']} at {g}"
                            )
                            continue
                        if not urgent:
                            break
                for item in deferred:
                    heapq.heappush(chains, item)

            for g in range(NSLOT + L + 1):
                if g < NSLOT:
                    emit_scores(g)
                for mul in pending_muls.pop(g, ()):
                    mul()
                lane_work(g)
                g2 = g - L
                if 0 <= g2 < NSLOT:
                    emit_attnv(g2)
                if g in norm_slot:
                    emit_norm_start(norm_slot[g], g)

            # drain remaining lane work (phase-3 of the last chunk)
            g = NSLOT + L + 1
            while chains or lanes[0]["pieces"] or lanes[1]["pieces"]:
                lane_work(g)
                g += 1

    nc.compile()
    return nc


def _get_nc():
    if "nc" not in _CACHE:
        _CACHE["nc"] = _build()
    return _CACHE["nc"]


def _prep_inputs(x, mask, Wq, Wk, Wv, Wo, bo):
    """Build the 8 per-core input maps (bf16 on the wire, p-major)."""
    import ml_dtypes

    bf16 = ml_dtypes.bfloat16
    x = np.asarray(x, dtype=np.float32)
    mask = np.asarray(mask, dtype=np.int32)
    wqT = np.asarray(Wq, np.float32).T
    wkT = np.asarray(Wk, np.float32).T
    wvT = np.asarray(Wv, np.float32).T
    woT = np.asarray(Wo, np.float32).T

    # x[b].T chunked: [NJ, 128, KT, 512] with xC[j, p, t, u] =
    # x[b].T[t*128+p, j*512+u]  (8KB per-partition contiguous runs)
    xCs = []
    for b in range(B):
        xT = x[b].T.astype(bf16)  # [D, S]
        xc = np.ascontiguousarray(
            xT.reshape(KT, 128, NJ, 512).transpose(2, 1, 0, 3)
        )
        xCs.append(xc)
    # keep-mask p-major: maskP[p, i, q] = (mask[b,0,q,i*128+p] == 0)
    maskPs = []
    for b in range(B):
        keepT = (mask[b, 0].T == 0).astype(bf16)  # [k, q]
        maskPs.append(
            np.ascontiguousarray(keepT.reshape(NSK, 128, S).transpose(1, 0, 2))
        )

    def wpm(wT, doff):  # [D, DPC] slice -> [128, KT, DPC]
        sl = np.ascontiguousarray(wT[:, doff : doff + DPC]).astype(bf16)
        return np.ascontiguousarray(sl.reshape(KT, 128, DPC).transpose(1, 0, 2))

    in_maps = []
    for c in range(NCORES):
        b, hq = c >> 2, c & 3
        doff = hq * DPC
        wos = np.ascontiguousarray(woT[doff : doff + DPC, :]).astype(bf16)
        in_maps.append(
            {
                "xC": xCs[b],
                "wqP": wpm(wqT, doff),
                "wkP": wpm(wkT, doff),
                "wvP": wpm(wvT, doff),
                "woP": np.ascontiguousarray(
                    wos.reshape(2, 128, D).transpose(1, 0, 2)
                ),
                "maskP": maskPs[b],
            }
        )
    return in_maps


def run(inputs: dict, trace: bool = False):
    """Run the kernel; returns (full_output, BassKernelResults)."""
    from concourse.bass_utils import run_bass_kernel_spmd

    nc = _get_nc()
    in_maps = _prep_inputs(**inputs)
    res = run_bass_kernel_spmd(
        nc, in_maps, core_ids=list(range(NCORES)), trace=trace
    )
    bo = np.asarray(inputs["bo"], dtype=np.float32)
    out = np.empty((B, S, D), dtype=np.float32)
    for b in range(B):
        acc = res.results[4 * b]["outp"].astype(np.float32)
        for hq in range(1, 4):
            acc = acc + res.results[4 * b + hq]["outp"].astype(np.float32)
        out[b] = acc + bo[None, :]
    return out, res


def kernel(**inputs) -> np.ndarray:
    out, _ = run(inputs, trace=False)
    return out


# revision 39
# speedup vs baseline: 1.4578x; 1.4578x over previous
"""Multi-head attention (B=2, S=2048, D=1024, H=16, Hd=64) on 8 Trainium2
NeuronCores.

Sharding: 8 cores = (batch 2) x (head-quarter 4).  Core (b, hq) computes,
for batch b and heads hq*4..hq*4+3, the full-sequence partial output

    outp = (softmax-attention of its 4 heads over all 2048 q rows) @ Wo_part.T

and the host sums the four head-quarter partials per batch and adds bo.

Everything is bf16 on the wire and in SBUF (PSUM accumulates fp32).
Host-side layouts are all p-major so every DMA descriptor is >= 4KB:
  xC     [NJ, 128, KT, 512]   x[b].T j-chunked, 8KB runs
  w{q,k,v}P [128, KT, DPC]    W.T slice, 4KB runs
  woP    [128, 2, D]          Wo.T slice, 4KB runs
  maskP  [128, NSK, S]        keep-mask (mask[b,0]==0).T, 4KB runs/tile
  outp   [S, D]               bf16 partial, 2KB rows

DMA queues: the 8MB mask streams on the qAct HWDGE queue (issued while
ACT is still idle) in parallel with x/weights/outputs on qSP.

Execution is one software-pipelined slot stream over all 8 (c, j) units
(c = head pair, j = 512-wide q chunk), 16 s_k tiles each — no
per-unit pipeline drain.  Slot g: scores matmul pair -> exp (ScalarE,
the pacing engine) -> keep-mask multiply (VectorE, 2x bf16 mode);
attnV accumulates L slots behind.  Projection chains, softmax-
normalization multiplies, and phase-3 output blocks are emitted from a
deadline-ordered (EDF) work heap, at most a few sub-microsecond pieces
per slot, so the PE stays dense without stalling the exp stream.
Emission order IS dependency order for the tile framework, so every
piece carries an exclusive deadline (= first slot that consumes it)
checked at build time.

Head packing: a pair's two heads live on partition halves 0-63/64-127
so the pair's two scores matmuls run concurrently on disjoint PE row
groups.  V lands in v_aug [128, NSK, head*128 + (64 V | 64 ones)]; the
ones columns make the attnV matmul accumulate Z = sum(expm) into PSUM
rows 64..127 at no extra stream cost (the stream time is set by the
512 moving columns, not the stationary width).
"""

import sys

if "/opt/trn_rl_repo" not in sys.path:
    sys.path.insert(0, "/opt/trn_rl_repo")

import heapq

import numpy as np

B, S, D = 2, 2048, 1024
H, HD = 16, 64
NCORES = 8
HPC = 4  # heads per core
DPC = HPC * HD  # 256 head dims per core
KT = D // 128  # 8 contraction tiles
NSK = S // 128  # 16 s_k tiles
NJ = S // 512  # 4 q chunks
NC2 = HPC // 2  # 2 head pairs

_CACHE = {}


def _build():
    import concourse.bacc as bacc
    import concourse.mybir as mybir
    import concourse.tile as tile

    F32 = mybir.dt.float32
    BF16 = mybir.dt.bfloat16
    MULT = mybir.AluOpType.mult
    EXP = mybir.ActivationFunctionType.Exp

    nc = bacc.Bacc("TRN2", target_bir_lowering=False, debug=False)

    xC = nc.dram_tensor("xC", [NJ, 128, KT, 512], BF16, kind="ExternalInput")
    wqP = nc.dram_tensor("wqP", [128, KT, DPC], BF16, kind="ExternalInput")
    wkP = nc.dram_tensor("wkP", [128, KT, DPC], BF16, kind="ExternalInput")
    wvP = nc.dram_tensor("wvP", [128, KT, DPC], BF16, kind="ExternalInput")
    woP = nc.dram_tensor("woP", [128, 2, D], BF16, kind="ExternalInput")
    maskP = nc.dram_tensor("maskP", [128, NSK, S], BF16, kind="ExternalInput")
    outp = nc.dram_tensor("outp", [S, D], BF16, kind="ExternalOutput")

    with tile.TileContext(nc) as tc:
        with (
            tc.tile_pool(name="keep", bufs=1) as keep,
            tc.tile_pool(name="pnorm", bufs=1) as pnorm,
            tc.tile_pool(name="p3s", bufs=2) as p3s,
            tc.tile_pool(name="scp", bufs=2, space="PSUM") as scp,
            tc.tile_pool(name="opp", bufs=1, space="PSUM") as opp,
            tc.tile_pool(name="aux0", bufs=1, space="PSUM") as aux0,
            tc.tile_pool(name="aux1", bufs=1, space="PSUM") as aux1,
        ):
            # ---- persistent SBUF ----------------------------------------
            x_sb = keep.tile([128, NJ, KT, 512], BF16)  # 32KB/part, j-major
            wq_sb = keep.tile([128, KT, DPC], BF16)
            wk_sb = keep.tile([128, KT, DPC], BF16)
            wv_sb = keep.tile([128, KT, DPC], BF16)
            wo_sb = keep.tile([128, 2, D], BF16)
            qT_sb = keep.tile([128, NC2, S], BF16)
            kT_sb = keep.tile([128, NC2, S], BF16)
            v_aug = keep.tile([128, NSK, HPC * 128], BF16)  # 16KB/part
            mask01 = keep.tile([128, NSK, S], BF16)  # 64KB/part, 0/1
            out_cT = keep.tile([128, NC2, S], BF16)
            # manual rings: exp output (6 slots; pairs at even offsets stay
            # adjacent) and masked-exp pairs (4 x 2-slot tiles) so one DVE
            # multiply covers two slots' tiles.
            expt_ring = keep.tile([128, 6, 2, 512], BF16)  # 24KB/part
            expm_ring = keep.tile([128, 4, 2, 2, 512], BF16)  # 16KB/part

            nc.any.memset(v_aug[:], 1.0)

            # ---- DMAs ---------------------------------------------------
            # Dual HWDGE queues (two DMA engines pull ~400GB/s aggregate
            # vs ~350 for one).  qAct front-loads the exp-stream critical
            # path: wk/x0/wq then mask tiles 0-7; its ring-credit blocking
            # of the ACT sequencer retires by ~10us, before the first exp.
            # qSP carries the rest in need-order plus the outputs.
            def dma_mask(engine, lo, hi):
                engine.dma_start(out=mask01[:, lo:hi, :], in_=maskP[:, lo:hi, :])

            def dma_x(engine, jc):
                engine.dma_start(out=x_sb[:, jc, :, :], in_=xC[jc, :, :, :])

            nc.scalar.dma_start(out=wk_sb[:], in_=wkP[:])
            dma_x(nc.scalar, 0)
            nc.scalar.dma_start(out=wq_sb[:], in_=wqP[:])
            dma_mask(nc.scalar, 0, 4)
            dma_mask(nc.scalar, 4, 8)

            nc.sync.dma_start(out=wv_sb[:], in_=wvP[:])
            dma_x(nc.sync, 1)
            dma_x(nc.sync, 2)
            dma_x(nc.sync, 3)
            dma_mask(nc.sync, 8, 12)
            dma_mask(nc.sync, 12, 16)
            nc.sync.dma_start(out=wo_sb[:], in_=woP[:])

            # ---- projection chains --------------------------------------
            # Every chain-like work item is a list of pieces, each a
            # function of the PSUM pool it accumulates in.  A piece list is
            # always emitted in order within ONE lane (one pool), so the
            # pool's single buffer is reused strictly
            # alloc->mm...->evict->alloc in emission order.
            def chain_kq(w_sb, dst_sb, c, jk, npieces=4):
                st = {}
                bounds = [
                    (KT * p // npieces, KT * (p + 1) // npieces)
                    for p in range(npieces)
                ]

                def piece(pool, r0, r1):
                    if r0 == 0:
                        st["ps"] = pool.tile(
                            [128, 512], F32, tag="aux", name=f"ch_{c}_{jk}"
                        )
                    ps = st["ps"]
                    for t in range(r0, r1):
                        nc.tensor.matmul(
                            ps[:],
                            w_sb[:, t, c * 128 : (c + 1) * 128],
                            x_sb[:, jk, t, :],
                            start=(t == 0),
                            stop=(t == KT - 1),
                        )
                    if r1 == KT:
                        sl = slice(jk * 512, (jk + 1) * 512)
                        nc.vector.tensor_copy(dst_sb[:, c, sl], ps[:])

                return [
                    lambda pool, r0=r0, r1=r1: piece(pool, r0, r1)
                    for r0, r1 in bounds
                ]

            def chain_v(sb):
                st = {}
                jv, uv = divmod(sb * 128, 512)

                def half(pool, r0, r1):
                    if r0 == 0:
                        st["ps"] = pool.tile(
                            [128, 256], F32, tag="aux", name=f"chv_{sb}"
                        )
                    ps = st["ps"]
                    for t in range(r0, r1):
                        nc.tensor.matmul(
                            ps[:],
                            x_sb[:, jv, t, uv : uv + 128],
                            wv_sb[:, t, :],
                            start=(t == 0),
                            stop=(t == KT - 1),
                        )
                    if r1 == KT:
                        nc.vector.tensor_copy(
                            v_aug[:, sb, :]
                            .rearrange("p (h c2) -> p h c2", h=HPC)[:, :, 0:HD],
                            ps[:].rearrange("p (h c2) -> p h c2", h=HPC),
                        )

                return [
                    lambda pool: half(pool, 0, 4),
                    lambda pool: half(pool, 4, KT),
                ]

            # ---- phase-3 output blocks ----------------------------------
            def phase3_block(mm, out_engine):
                """One m-block as a 4-piece chain: (n half x cb step); the
                last piece of each n half evicts, the last DMAs the row."""
                msl = slice(mm * 128, (mm + 1) * 128)
                st = {}

                def step(pool, cb, n):
                    if cb == 0:
                        st["ps"] = pool.tile(
                            [128, 512], F32, tag="aux", name=f"p3_{mm}_{n}"
                        )
                    nc.tensor.matmul(
                        st["ps"][:],
                        out_cT[:, cb, msl],
                        wo_sb[:, cb, n * 512 : (n + 1) * 512],
                        start=(cb == 0),
                        stop=(cb == 1),
                    )
                    if cb == 1:
                        if n == 0:
                            st["ob"] = p3s.tile(
                                [128, 1024], BF16, tag="ob", name=f"ob_{mm}"
                            )
                        nc.vector.tensor_copy(
                            st["ob"][:, n * 512 : (n + 1) * 512], st["ps"][:]
                        )
                        if n == 1:
                            out_engine.dma_start(
                                out=outp[msl, :], in_=st["ob"][:]
                            )

                return [
                    lambda pool, cb=cb, n=n: step(pool, cb, n)
                    for n in range(2)
                    for cb in range(2)
                ]

            # ---- the pipelined slot stream ------------------------------
            # Hybrid unit order: units 0-2 need only prefix + light Q
            # chains; K(1,*) has a 45-slot runway to unit 3; phase-3 of
            # j=0/1/2 spreads from units 4/6/7 onward.
            L = 6  # attnV lookahead in slots
            UNITS = [(0, 0), (0, 1), (1, 0), (0, 2), (1, 1), (0, 3), (1, 2), (1, 3)]
            NU = len(UNITS)
            NSLOT = NU * NSK
            U_OF = {cj: u for u, cj in enumerate(UNITS)}

            # PE warm-up: a dozen throwaway matmuls on memset scratch keep
            # the PE pstate ramping while the first DMAs land, so the
            # prefix chains run at mid clock instead of 0.65GHz.
            warm = keep.tile([128, 512], BF16)
            nc.vector.memset(warm[:], 1.0)
            wps = aux0.tile([128, 512], F32, tag="aux", name="warm_ps")
            for _ in range(12):
                nc.tensor.matmul(
                    wps[:], warm[:, 0:128], warm[:], start=True, stop=True
                )

            # prefix: only what gates the first few slots; everything else
            # streams in as lane work during unit 0, whose pace is set by
            # mask-DMA arrival anyway.  Alternate pools so consecutive
            # chains overlap (eviction of one vs matmuls of the next).
            prefix = [
                (chain_kq(wk_sb, kT_sb, 0, 0, 1), aux1),
                (chain_kq(wq_sb, qT_sb, 0, 0, 1), aux0),
                (chain_v(0), aux1),
                (chain_v(1), aux0),
            ]
            for pieces, pool in prefix:
                for p in pieces:
                    p(pool)

            # Chain heap: (first_sdl, seq, ready_slot, true_dl, pieces)
            # where pieces = [(sdl, fn), ...].  Two lanes, one PSUM pool
            # each; a lane hosts one chain at a time and emits a piece only
            # once its scheduled slot arrives (or the chain's true deadline
            # forces a burst).  The per-piece schedule drip-feeds the PE
            # uniformly — a bunched chain blob at a dropped PE clock is
            # what stalled the exp stream ~6us every unit.  A chain must
            # finish emission at a slot < true_dl; asserted below.
            chains = []
            _seq = [0]

            def add(deadline, fns, ready=0, spacing=1, end_slack=1, true_dl=None):
                n = len(fns)
                pieces = [
                    (max(deadline - end_slack - spacing * (n - 1 - i), 0), fn)
                    for i, fn in enumerate(fns)
                ]
                heapq.heappush(
                    chains,
                    (
                        pieces[0][0],
                        _seq[0],
                        ready,
                        true_dl if true_dl is not None else deadline,
                        pieces,
                    ),
                )
                _seq[0] += 1

            # remaining V chains: v(sb) consumed by attnV(0, sb) at slot
            # sb+L (lane work precedes attnV within a slot -> dl sb+L+1)
            for sb in range(2, NSK):
                add(sb + L + 1, chain_v(sb))
            # K(0,jk) consumed by scores(0, 4jk) at slot 4jk
            for jk in range(1, 4):
                add(4 * jk, chain_kq(wk_sb, kT_sb, 0, jk))
            # Q and K(1,*) chains, scheduled into slots i=0..6 of the
            # preceding unit (i=7..15 carry the phase-3 drip; overlapping
            # both in one slot is what stalled the exp at i~12)
            def slack_for(dl):
                # slots 0..~22 are fully booked by v/K(0,*) chains; only
                # chains due from slot 48 on can shift into the i=0..6
                # window of their preceding unit
                return 10 if dl >= 3 * NSK else 4

            def add_kq(dl, w_sb, dst_sb, c, jk):
                add(dl, chain_kq(w_sb, dst_sb, c, jk),
                    spacing=2, end_slack=slack_for(dl))

            for j in range(1, 4):
                add_kq(NSK * U_OF[(0, j)], wq_sb, qT_sb, 0, j)
            u10 = U_OF[(1, 0)]
            add_kq(NSK * u10, wq_sb, qT_sb, 1, 0)
            for jk in range(4):
                add_kq(NSK * u10 + 4 * jk, wk_sb, kT_sb, 1, jk)
            for j in range(1, 4):
                add_kq(NSK * U_OF[(1, j)], wq_sb, qT_sb, 1, j)
            # phase-3 m-blocks: ready once both (0,j) and (1,j) norm
            # multiplies are emitted; no real deadline (drain at the end).
            # The drain chunk's outputs ride the by-then-idle qAct queue.
            for j in range(4):
                gate = max(U_OF[(0, j)], U_OF[(1, j)])
                ready = gate * NSK + NSK - 1 + L + 2
                out_engine = nc.scalar if gate == NU - 1 else nc.sync
                for m in range(4):
                    add(
                        ready + 4 * m + 4,
                        phase3_block(4 * j + m, out_engine),
                        ready=ready,
                        true_dl=10**6,
                    )

            out_ps = [None] * NU

            def emit_scores(g):
                u, i = divmod(g, NSK)
                c, j = UNITS[u]
                jsl = slice(j * 512, (j + 1) * 512)
                sc = scp.tile([128, 2, 512], F32, tag="sc", name=f"sc_{u}_{i}")
                for h2 in range(2):
                    hsl = slice(h2 * 64, (h2 + 1) * 64)
                    nc.tensor.matmul(
                        sc[:, h2, :],
                        kT_sb[hsl, c, i * 128 : (i + 1) * 128],
                        qT_sb[hsl, c, jsl],
                        start=True,
                        stop=True,
                    )
                nc.scalar.activation(
                    out=expt_ring[:, g % 6, :, :], in_=sc[:], func=EXP, scale=0.125
                )
                # keep-mask multiply as slot-pair DVE ops (gpsimd TT was
                # measured ~5x slower than its cost model — do not offload)
                if g % 2 == 1:
                    r0 = (g - 1) % 6
                    nc.vector.tensor_tensor(
                        out=expm_ring[:, (g // 2) % 4, :, :, :],
                        in0=expt_ring[:, r0 : r0 + 2, :, :],
                        in1=mask01[:, i - 1 : i + 1, jsl][
                            :, :, None, :
                        ].to_broadcast((128, 2, 2, 512)),
                        op=MULT,
                    )

            def emit_attnv(g2):
                u, i = divmod(g2, NSK)
                c, j = UNITS[u]
                if i == 0:
                    out_ps[u] = opp.tile(
                        [128, 2, 512], F32, tag="ops", name=f"op_{u}"
                    )
                for h2 in range(2):
                    h = 2 * c + h2
                    nc.tensor.matmul(
                        out_ps[u][:, h2, :],
                        v_aug[:, i, h * 128 : (h + 1) * 128],
                        expm_ring[:, (g2 // 2) % 4, g2 % 2, h2, :],
                        start=(i == 0),
                        stop=(i == NSK - 1),
                    )

            pending_muls = {}

            def emit_norm_start(u, g):
                """Z reciprocal + gpsimd broadcast right after unit u's last
                attnV; the two DVE multiplies are emitted directly at slot
                g+1, before the next unit's first attnV (which reuses the
                single opp buffer)."""
                c, j = UNITS[u]
                jsl = slice(j * 512, (j + 1) * 512)
                # both heads' Z rows live on partition 64 (h2 is a free
                # dim), so one copy + one reciprocal covers the pair
                zrow = pnorm.tile([1, 2, 512], F32, tag="zrow", name=f"zw_{u}")
                nc.vector.tensor_copy(zrow[:], out_ps[u][64:65, :, :])
                zr1 = pnorm.tile([1, 2, 512], F32, tag="zr1", name=f"z1_{u}")
                nc.vector.reciprocal_approx_fast(out=zr1[:], in_=zrow[:])
                muls = []
                for h2 in range(2):
                    zr = pnorm.tile([64, 512], F32, tag="zr", name=f"zr_{u}_{h2}")
                    nc.gpsimd.partition_broadcast(zr[:], zr1[:, h2, :])

                    def mul(u=u, c=c, h2=h2, jsl=jsl, zr=zr):
                        nc.vector.tensor_tensor(
                            out=out_cT[h2 * 64 : (h2 + 1) * 64, c, jsl],
                            in0=out_ps[u][0:64, h2, :],
                            in1=zr[:],
                            op=MULT,
                        )

                    muls.append(mul)
                pending_muls[g + 1] = muls

            norm_slot = {u * NSK + NSK - 1 + L: u for u in range(NU)}

            lanes = [
                {"pool": aux0, "pieces": [], "true_dl": 0},
                {"pool": aux1, "pieces": [], "true_dl": 0},
            ]

            def lane_work(g):
                deferred = []
                for lane in lanes:
                    emitted = 0
                    while emitted < 3:
                        if not lane["pieces"]:
                            got = None
                            while chains:
                                item = heapq.heappop(chains)
                                if item[2] > g:  # not ready yet
                                    deferred.append(item)
                                    continue
                                got = item
                                break
                            if got is None:
                                break
                            _sdl, _sq, _rdy, true_dl, pieces = got
                            lane["pieces"] = pieces
                            lane["true_dl"] = true_dl
                        # emit the head piece if its slot has come, or the
                        # chain's true deadline forces a burst
                        sdl, fn = lane["pieces"][0]
                        urgent = lane["true_dl"] <= g + 1 + len(lane["pieces"])
                        if sdl > g and not urgent:
                            break
                        lane["pieces"].pop(0)
                        fn(lane["pool"])
                        emitted += 1
                        if not lane["pieces"]:
                            assert lane["true_dl"] > g, (
                                f"chain past deadline {lane['true_dl']} at {g}"
                            )
                            continue
                        if not urgent:
                            break
                for item in deferred:
                    heapq.heappush(chains, item)

            for g in range(NSLOT + L + 1):
                if g < NSLOT:
                    emit_scores(g)
                for mul in pending_muls.pop(g, ()):
                    mul()
                lane_work(g)
                g2 = g - L
                if 0 <= g2 < NSLOT:
                    emit_attnv(g2)
                if g in norm_slot:
                    emit_norm_start(norm_slot[g], g)

            # drain remaining lane work (phase-3 of the last chunk)
            g = NSLOT + L + 1
            while chains or lanes[0]["pieces"] or lanes[1]["pieces"]:
                lane_work(g)
                g += 1

    nc.compile()
    return nc


def _get_nc():
    if "nc" not in _CACHE:
        _CACHE["nc"] = _build()
    return _CACHE["nc"]


def _prep_inputs(x, mask, Wq, Wk, Wv, Wo, bo):
    """Build the 8 per-core input maps (bf16 on the wire, p-major)."""
    import ml_dtypes

    bf16 = ml_dtypes.bfloat16
    x = np.asarray(x, dtype=np.float32)
    mask = np.asarray(mask, dtype=np.int32)
    wqT = np.asarray(Wq, np.float32).T
    wkT = np.asarray(Wk, np.float32).T
    wvT = np.asarray(Wv, np.float32).T
    woT = np.asarray(Wo, np.float32).T

    # x[b].T chunked: [NJ, 128, KT, 512] with xC[j, p, t, u] =
    # x[b].T[t*128+p, j*512+u]  (8KB per-partition contiguous runs)
    xCs = []
    for b in range(B):
        xT = x[b].T.astype(bf16)  # [D, S]
        xc = np.ascontiguousarray(
            xT.reshape(KT, 128, NJ, 512).transpose(2, 1, 0, 3)
        )
        xCs.append(xc)
    # keep-mask p-major: maskP[p, i, q] = (mask[b,0,q,i*128+p] == 0)
    maskPs = []
    for b in range(B):
        keepT = (mask[b, 0].T == 0).astype(bf16)  # [k, q]
        maskPs.append(
            np.ascontiguousarray(keepT.reshape(NSK, 128, S).transpose(1, 0, 2))
        )

    def wpm(wT, doff):  # [D, DPC] slice -> [128, KT, DPC]
        sl = np.ascontiguousarray(wT[:, doff : doff + DPC]).astype(bf16)
        return np.ascontiguousarray(sl.reshape(KT, 128, DPC).transpose(1, 0, 2))

    in_maps = []
    for c in range(NCORES):
        b, hq = c >> 2, c & 3
        doff = hq * DPC
        wos = np.ascontiguousarray(woT[doff : doff + DPC, :]).astype(bf16)
        in_maps.append(
            {
                "xC": xCs[b],
                "wqP": wpm(wqT, doff),
                "wkP": wpm(wkT, doff),
                "wvP": wpm(wvT, doff),
                "woP": np.ascontiguousarray(
                    wos.reshape(2, 128, D).transpose(1, 0, 2)
                ),
                "maskP": maskPs[b],
            }
        )
    return in_maps


def run(inputs: dict, trace: bool = False):
    """Run the kernel; returns (full_output, BassKernelResults)."""
    from concourse.bass_utils import run_bass_kernel_spmd

    nc = _get_nc()
    in_maps = _prep_inputs(**inputs)
    res = run_bass_kernel_spmd(
        nc, in_maps, core_ids=list(range(NCORES)), trace=trace
    )
    bo = np.asarray(inputs["bo"], dtype=np.float32)
    out = np.empty((B, S, D), dtype=np.float32)
    for b in range(B):
        acc = res.results[4 * b]["outp"].astype(np.float32)
        for hq in range(1, 4):
            acc = acc + res.results[4 * b + hq]["outp"].astype(np.float32)
        out[b] = acc + bo[None, :]
    return out, res


def kernel(**inputs) -> np.ndarray:
    out, _ = run(inputs, trace=False)
    return out
